# revision 11
# baseline (speedup 1.0000x reference)
"""GNN encoder (SAGEConv + 2x GINConv + BN + global_add_pool) on 8 TRN2 NeuronCores.

Self-contained SPMD Bass kernel. Sharding: destination-node ranges (12500 nodes
per core) with their incident edges; message gather via dma_gather from a
replicated node-feature table; segment-sum via one-hot matmuls accumulated in
PSUM + SBUF; BN stats all-reduced across cores; h re-replicated between layers
with AllGather; per-graph pooling via one-hot matmul, combined on host.
"""
import math
import numpy as np
from contextlib import ExitStack

import concourse.bass as bass
import concourse.bacc as bacc
import concourse.tile as tile
from concourse import mybir
from concourse.bass_utils import run_bass_kernel_spmd

# problem constants (hardcoded per contract; configure() exists for small-scale tests)
F, D, L = 128, 128, 3
BN_EPS = 1e-5
NCORES = 8
F32 = mybir.dt.float32
I16 = mybir.dt.int16


def configure(n=100000, e=1600000, num_graphs=500, buck=32768, call_chunks=8):
    global N, E, NUM_GRAPHS, NC, NT, NCPAD, TROWS, BUCK, NBUCK, CALL_CHUNKS
    N, E, NUM_GRAPHS, BUCK, CALL_CHUNKS = n, e, num_graphs, buck, call_chunks
    NC = N // NCORES
    NT = math.ceil(NC / 128)
    NCPAD = NT * 128
    TROWS = NCORES * NCPAD
    NBUCK = math.ceil(TROWS / BUCK)


configure()


def _wrap16(flat):
    """[n*16] -> [128, n] int16 'wrapped in 16 partitions, replicated x8'."""
    w = flat.reshape(-1, 16).T.astype(np.int16)   # [16, n]
    return np.tile(w, (8, 1)).copy()


def _preprocess(x, edge_index, batch):
    src = edge_index[0].astype(np.int64)
    dst = edge_index[1].astype(np.int64)
    row = (src // NC) * NCPAD + (src % NC)        # padded table row per source
    bkt = row // BUCK

    # per-(core, tile, bucket) counts -> uniform chunk table M
    cnts = np.zeros((NCORES, NT, NBUCK), np.int64)
    core_of = dst // NC
    ld_all = dst - core_of * NC
    t_all = ld_all // 128
    key_all = t_all * NBUCK + bkt
    for c in range(NCORES):
        sel = core_of == c
        cnts[c] = np.bincount(key_all[sel], minlength=NT * NBUCK).reshape(NT, NBUCK)
    mx = cnts.max(axis=0)
    M = (mx + 127) // 128                          # [NT, NBUCK] chunks
    off_b = np.zeros((NT + 1, NBUCK), np.int64)    # chunk offset within bucket
    off_b[1:] = np.cumsum(M, axis=0)
    C_b = off_b[-1]                                # chunks per bucket
    bucket_cum = np.zeros(NBUCK + 1, np.int64)
    bucket_cum[1:] = np.cumsum(C_b)
    CTOT = int(bucket_cum[-1])

    # per-key slot base (in edge slots): (bucket_cum[b] + off_b[t,b]) * 128
    slot_base = (bucket_cum[None, :NBUCK] + off_b[:NT]) * 128  # [NT, NBUCK]

    deg = np.bincount(dst, minlength=N).astype(np.float64)
    invdeg_full = (1.0 / np.maximum(deg, 1.0)).astype(np.float32)

    graph_base = [int(batch[c * NC]) for c in range(NCORES)]
    for c in range(NCORES):
        span = int(batch[(c + 1) * NC - 1]) - graph_base[c] + 1
        assert span <= 128, f"graph span {span} exceeds 128"

    per_core = []
    for c in range(NCORES):
        sel = core_of == c
        ks = key_all[sel]
        rows = row[sel]
        dl = ld_all[sel] % 128
        order = np.argsort(ks, kind="stable")
        ksort = ks[order]
        gstart = np.searchsorted(ksort, np.arange(NT * NBUCK), side="left")
        rank = np.arange(len(ksort)) - gstart[ksort]
        slot = slot_base.reshape(-1)[ksort] + rank
        idxflat = np.zeros(CTOT * 128, np.int16)
        dstlflat = np.full(CTOT * 128, -1.0, np.float32)
        idxflat[slot] = (rows[order] - (ksort % NBUCK) * BUCK).astype(np.int16)
        dstlflat[slot] = dl[order].astype(np.float32)

        gidx = [
            _wrap16(idxflat[bucket_cum[b] * 128:bucket_cum[b + 1] * 128])
            for b in range(NBUCK)
        ]
        dstl = dstlflat.reshape(CTOT, 128).T.copy()   # [128, CTOT]

        xT = np.zeros((128, NCPAD), np.float32)
        xT[:, :NC] = x[c * NC:(c + 1) * NC].T
        invd = np.zeros(NCPAD, np.float32)
        invd[:NC] = invdeg_full[c * NC:(c + 1) * NC]
        invdeg = invd.reshape(NT, 128).T.copy()       # [128, NT]
        bs = np.full(NCPAD, -1.0, np.float32)
        bs[:NC] = (batch[c * NC:(c + 1) * NC] - graph_base[c]).astype(np.float32)
        batch_sel = bs.reshape(NT, 128).T.copy()      # [128, NT]
        per_core.append(dict(gidx=gidx, dstl=dstl, xT=xT, invdeg=invdeg,
                             batch_sel=batch_sel))

    x_table = np.zeros((TROWS, 128), np.float32)
    for c in range(NCORES):
        x_table[c * NCPAD:c * NCPAD + NC] = x[c * NC:(c + 1) * NC]

    meta = dict(M=M, off_b=off_b, C_b=C_b, bucket_cum=bucket_cum, CTOT=CTOT)
    return meta, per_core, x_table, graph_base


def _build(meta):
    M = meta["M"]; off_b = meta["off_b"]; C_b = meta["C_b"]
    bucket_cum = meta["bucket_cum"]; CTOT = meta["CTOT"]
    bucket_base = [b * BUCK for b in range(NBUCK)]
    bucket_size = [min(BUCK, TROWS - b * BUCK) for b in range(NBUCK)]

    nc = bacc.Bacc(None, target_bir_lowering=False, debug=False, num_devices=NCORES)

    x_table = nc.dram_tensor("x_table", [TROWS, 128], F32, kind="ExternalInput")
    xT = nc.dram_tensor("xT", [128, NCPAD], F32, kind="ExternalInput")
    dstl_d = nc.dram_tensor("dstl", [128, CTOT], F32, kind="ExternalInput")
    gidx_d = [nc.dram_tensor(f"gidx{b}", [128, int(C_b[b]) * 8], I16, kind="ExternalInput")
              for b in range(NBUCK)]
    invdeg_d = nc.dram_tensor("invdeg", [128, NT], F32, kind="ExternalInput")
    batch_d = nc.dram_tensor("batch_sel", [128, NT], F32, kind="ExternalInput")
    iota_d = nc.dram_tensor("iota", [128, 128], F32, kind="ExternalInput")
    ident_d = nc.dram_tensor("ident", [128, 128], F32, kind="ExternalInput")
    w_d = nc.dram_tensor("weights", [128, 6 * 128], F32, kind="ExternalInput")
    bias_d = nc.dram_tensor("biases", [128, 5], F32, kind="ExternalInput")
    gamma_d = nc.dram_tensor("gamma", [128, L], F32, kind="ExternalInput")
    beta_d = nc.dram_tensor("beta", [128, L], F32, kind="ExternalInput")

    node_feats = nc.dram_tensor("node_feats", [NCPAD, L * 128], F32, kind="ExternalOutput")
    pooled = nc.dram_tensor("pooled", [128, L * 128], F32, kind="ExternalOutput")

    with tile.TileContext(nc) as tc, ExitStack() as ctx:
        const = ctx.enter_context(tc.tile_pool(name="const", bufs=1))
        pbig = ctx.enter_context(tc.tile_pool(name="big", bufs=1))
        pg = ctx.enter_context(tc.tile_pool(name="gath", bufs=3))
        pgi = ctx.enter_context(tc.tile_pool(name="gidx", bufs=3))
        psel = ctx.enter_context(tc.tile_pool(name="sel", bufs=4))
        ptmp = ctx.enter_context(tc.tile_pool(name="tmp", bufs=3))
        pst = ctx.enter_context(tc.tile_pool(name="st", bufs=2))
        ppmm = ctx.enter_context(tc.tile_pool(name="pmm", bufs=2, space="PSUM"))
        ppool = ctx.enter_context(tc.tile_pool(name="ppool", bufs=2, space="PSUM"))
        dram = ctx.enter_context(tc.tile_pool(name="dram", bufs=1, space="DRAM"))

        # constants
        iota_sb = const.tile([128, 128], F32)
        nc.sync.dma_start(out=iota_sb[:], in_=iota_d[:, :])
        ident_sb = const.tile([128, 128], F32)
        nc.sync.dma_start(out=ident_sb[:], in_=ident_d[:, :])
        dstl_sb = const.tile([128, CTOT], F32)
        nc.sync.dma_start(out=dstl_sb[:], in_=dstl_d[:, :])
        invdeg_sb = const.tile([128, NT], F32)
        nc.sync.dma_start(out=invdeg_sb[:], in_=invdeg_d[:, :])
        batch_sb = const.tile([128, NT], F32)
        nc.sync.dma_start(out=batch_sb[:], in_=batch_d[:, :])
        w_sb = const.tile([128, 6 * 128], F32)
        nc.sync.dma_start(out=w_sb[:], in_=w_d[:, :])
        bias_sb = const.tile([128, 5], F32)
        nc.sync.dma_start(out=bias_sb[:], in_=bias_d[:, :])
        gamma_sb = const.tile([128, L], F32)
        nc.sync.dma_start(out=gamma_sb[:], in_=gamma_d[:, :])
        beta_sb = const.tile([128, L], F32)
        nc.sync.dma_start(out=beta_sb[:], in_=beta_d[:, :])
        eps_sb = const.tile([128, 1], F32)
        nc.vector.memset(eps_sb[:], BN_EPS)

        def wslice(i):   # weight matrix i in [Wl, Wr, W1_0, W2_0, W1_1, W2_1]
            return w_sb[:, i * 128:(i + 1) * 128]

        hA = pbig.tile([128, NCPAD], F32, tag="hA")
        nc.sync.dma_start(out=hA[:], in_=xT[:, :])
        hB = pbig.tile([128, NCPAD], F32, tag="hB")
        aggr = pbig.tile([128, NCPAD], F32, tag="aggr")

        h_tab = [None] + [dram.tile([TROWS, 128], F32, tag=f"htab{l}", name=f"htab{l}", addr_space="Shared") for l in (1, 2)]
        h_loc = [dram.tile([NCPAD, 128], F32, tag=f"hloc{l}", name=f"hloc{l}") for l in (0, 1)]
        ar_in = [dram.tile([128, 2], F32, tag=f"arin{l}", name=f"arin{l}") for l in range(L)]
        ar_out = [dram.tile([128, 2], F32, tag=f"arout{l}", name=f"arout{l}", addr_space="Shared") for l in range(L)]

        for l in range(L):
            cur, prev = (hB, hA) if l % 2 == 0 else (hA, hB)
            table_src = x_table if l == 0 else h_tab[l]

            # ---------- aggregation: aggr = segment_sum over incident edges
            nc.vector.memset(aggr[:], 0.0)
            for b in range(NBUCK):
                gtiles = {}
                ncalls = (int(C_b[b]) + CALL_CHUNKS - 1) // CALL_CHUNKS
                for t in range(NT):
                    mtb = int(M[t, b])
                    if mtb == 0:
                        continue
                    ps = ppmm.tile([128, 128], F32, tag="segps")
                    for j in range(mtb):
                        lc = int(off_b[t, b]) + j
                        k, s = lc // CALL_CHUNKS, lc % CALL_CHUNKS
                        if k not in gtiles:
                            nch = min(CALL_CHUNKS, int(C_b[b]) - k * CALL_CHUNKS)
                            gt = pgi.tile([128, CALL_CHUNKS * 8], I16, tag="gidx")
                            nc.sync.dma_start(
                                out=gt[:, :nch * 8],
                                in_=gidx_d[b][:, k * CALL_CHUNKS * 8:
                                              (k * CALL_CHUNKS + nch) * 8])
                            gg = pg.tile([128, CALL_CHUNKS, 128], F32, tag="gath")
                            nc.gpsimd.dma_gather(
                                out_ap=gg[:, :nch, :],
                                in_ap=table_src[bucket_base[b]:
                                                bucket_base[b] + bucket_size[b], :],
                                idxs_ap=gt[:, :nch * 8],
                                num_idxs=nch * 128,
                                num_idxs_reg=nch * 128,
                                elem_size=128,
                            )
                            gtiles[k] = gg
                        cg = int(bucket_cum[b]) + lc
                        S = psel.tile([128, 128], F32, tag="S")
                        nc.vector.tensor_tensor(
                            out=S[:],
                            in0=dstl_sb[:, cg:cg + 1].to_broadcast([128, 128]),
                            in1=iota_sb[:],
                            op=mybir.AluOpType.is_equal)
                        g_chunk = gtiles[k][:, s, :]
                        if l == 0:   # node-major [d, f] (for per-node invdeg scale)
                            nc.tensor.matmul(out=ps[:], lhsT=S[:], rhs=g_chunk,
                                             start=(j == 0), stop=(j == mtb - 1))
                        else:        # feature-major [f, d]
                            nc.tensor.matmul(out=ps[:], lhsT=g_chunk, rhs=S[:],
                                             start=(j == 0), stop=(j == mtb - 1))
                    tsl = aggr[:, t * 128:(t + 1) * 128]
                    nc.vector.tensor_add(out=tsl, in0=tsl, in1=ps[:])

            # ---------- linear layer per dst tile
            for t in range(NT):
                asl = aggr[:, t * 128:(t + 1) * 128]
                csl = cur[:, t * 128:(t + 1) * 128]
                if l == 0:
                    sc = ptmp.tile([128, 128], F32, tag="sc")
                    nc.scalar.activation(out=sc[:], in_=asl,
                                         func=mybir.ActivationFunctionType.Copy,
                                         scale=invdeg_sb[:, t:t + 1])
                    trp = ppmm.tile([128, 128], F32, tag="trp")
                    nc.tensor.transpose(out=trp[:], in_=sc[:], identity=ident_sb[:])
                    trs = ptmp.tile([128, 128], F32, tag="trs")
                    nc.vector.tensor_copy(out=trs[:], in_=trp[:])
                    hp = ppmm.tile([128, 128], F32, tag="linp")
                    nc.tensor.matmul(out=hp[:], lhsT=wslice(0), rhs=trs[:],
                                     start=True, stop=False)
                    nc.tensor.matmul(out=hp[:], lhsT=wslice(1),
                                     rhs=hA[:, t * 128:(t + 1) * 128],
                                     start=False, stop=True)
                    nc.scalar.activation(out=csl, in_=hp[:],
                                         func=mybir.ActivationFunctionType.Relu,
                                         bias=bias_sb[:, 0:1])
                else:
                    t0 = ptmp.tile([128, 128], F32, tag="t0")
                    nc.vector.tensor_add(out=t0[:], in0=asl,
                                         in1=prev[:, t * 128:(t + 1) * 128])
                    p1 = ppmm.tile([128, 128], F32, tag="linp")
                    nc.tensor.matmul(out=p1[:], lhsT=wslice(2 * l), rhs=t0[:],
                                     start=True, stop=True)
                    t1 = ptmp.tile([128, 128], F32, tag="t1")
                    nc.scalar.activation(out=t1[:], in_=p1[:],
                                         func=mybir.ActivationFunctionType.Relu,
                                         bias=bias_sb[:, 2 * l - 1:2 * l])
                    p2 = ppmm.tile([128, 128], F32, tag="linp")
                    nc.tensor.matmul(out=p2[:], lhsT=wslice(2 * l + 1), rhs=t1[:],
                                     start=True, stop=True)
                    nc.scalar.activation(out=csl, in_=p2[:],
                                         func=mybir.ActivationFunctionType.Relu,
                                         bias=bias_sb[:, 2 * l:2 * l + 1])

            # ---------- BN stats over valid nodes (12500), cross-core allreduce
            sw = next(w for w in range(min(512, NC), 0, -2) if NC % w == 0)
            nslc = [(k * sw, sw) for k in range(NC // sw)]
            stats = pst.tile([128, len(nslc), 6], F32, tag="stats")
            for k, (s0, sl) in enumerate(nslc):
                nc.vector.bn_stats(out=stats[:, k, :], in_=cur[:, s0:s0 + sl])
            mv = pst.tile([128, 2], F32, tag="mv")
            nc.vector.bn_aggr(out=mv[:], in_=stats[:])
            tmp1 = pst.tile([128, 1], F32, tag="tmp1")
            nc.vector.tensor_mul(out=tmp1[:], in0=mv[:, 0:1], in1=mv[:, 0:1])
            ex2 = pst.tile([128, 1], F32, tag="ex2")
            nc.vector.tensor_add(out=ex2[:], in0=mv[:, 1:2], in1=tmp1[:])
            pk = pst.tile([128, 2], F32, tag="pk")
            nc.vector.tensor_scalar_mul(out=pk[:, 0:1], in0=mv[:, 0:1],
                                        scalar1=1.0 / NCORES)
            nc.vector.tensor_scalar_mul(out=pk[:, 1:2], in0=ex2[:],
                                        scalar1=1.0 / NCORES)
            nc.gpsimd.dma_start(out=ar_in[l][:], in_=pk[:])
            nc.gpsimd.collective_compute(
                "AllReduce", mybir.AluOpType.add,
                replica_groups=[list(range(NCORES))],
                ins=[ar_in[l][:].opt()], outs=[ar_out[l][:].opt()])
            gst = pst.tile([128, 2], F32, tag="gst")
            nc.sync.dma_start(out=gst[:], in_=ar_out[l][:])
            gm2 = pst.tile([128, 1], F32, tag="gm2")
            nc.vector.tensor_mul(out=gm2[:], in0=gst[:, 0:1], in1=gst[:, 0:1])
            gvar = pst.tile([128, 1], F32, tag="gvar")
            nc.vector.tensor_tensor(out=gvar[:], in0=gst[:, 1:2], in1=gm2[:],
                                    op=mybir.AluOpType.subtract)
            sq = pst.tile([128, 1], F32, tag="sq")
            nc.scalar.activation(out=sq[:], in_=gvar[:],
                                 func=mybir.ActivationFunctionType.Sqrt,
                                 bias=eps_sb[:])
            rs = pst.tile([128, 1], F32, tag="rs")
            nc.vector.reciprocal(out=rs[:], in_=sq[:])
            a_sc = pst.tile([128, 1], F32, tag="a_sc")
            nc.vector.tensor_mul(out=a_sc[:], in0=rs[:], in1=gamma_sb[:, l:l + 1])
            t3 = pst.tile([128, 1], F32, tag="t3")
            nc.vector.tensor_mul(out=t3[:], in0=gst[:, 0:1], in1=a_sc[:])
            b_sc = pst.tile([128, 1], F32, tag="b_sc")
            nc.vector.tensor_tensor(out=b_sc[:], in0=beta_sb[:, l:l + 1], in1=t3[:],
                                    op=mybir.AluOpType.subtract)

            # ---------- BN apply + transpose + outputs + pooling
            pooled_ps = ppool.tile([128, 128], F32, tag="pooled")
            for t in range(NT):
                csl = cur[:, t * 128:(t + 1) * 128]
                nc.vector.tensor_scalar(out=csl, in0=csl,
                                        scalar1=a_sc[:], scalar2=b_sc[:],
                                        op0=mybir.AluOpType.mult,
                                        op1=mybir.AluOpType.add)
                trp = ppmm.tile([128, 128], F32, tag="trp")
                nc.tensor.transpose(out=trp[:], in_=csl, identity=ident_sb[:])
                hbnT = ptmp.tile([128, 128], F32, tag="hbnT")
                nc.vector.tensor_copy(out=hbnT[:], in_=trp[:])
                nc.sync.dma_start(
                    out=node_feats[t * 128:(t + 1) * 128, l * 128:(l + 1) * 128],
                    in_=hbnT[:])
                if l < L - 1:
                    nc.sync.dma_start(out=h_loc[l][t * 128:(t + 1) * 128, :],
                                      in_=hbnT[:])
                selG = psel.tile([128, 128], F32, tag="S")
                nc.vector.tensor_tensor(
                    out=selG[:],
                    in0=batch_sb[:, t:t + 1].to_broadcast([128, 128]),
                    in1=iota_sb[:],
                    op=mybir.AluOpType.is_equal)
                nc.tensor.matmul(out=pooled_ps[:], lhsT=selG[:], rhs=hbnT[:],
                                 start=(t == 0), stop=(t == NT - 1))
            pooled_sb = ptmp.tile([128, 128], F32, tag="pooled_sb")
            nc.vector.tensor_copy(out=pooled_sb[:], in_=pooled_ps[:])
            nc.sync.dma_start(out=pooled[:, l * 128:(l + 1) * 128], in_=pooled_sb[:])

            if l < L - 1:
                nc.gpsimd.collective_compute(
                    "AllGather", mybir.AluOpType.bypass,
                    replica_groups=[list(range(NCORES))],
                    ins=[h_loc[l][:].opt()], outs=[h_tab[l + 1][:].opt()])

    nc.compile()
    return nc


def prepare(x, edge_index, batch, sage_Wl, sage_bl, sage_Wr,
            gin_W1, gin_b1, gin_W2, gin_b2, bn_gamma, bn_beta):
    x = np.asarray(x, np.float32)
    edge_index = np.asarray(edge_index)
    batch = np.asarray(batch)
    meta, per_core, x_table, graph_base = _preprocess(x, edge_index, batch)

    weights = np.concatenate(
        [np.asarray(sage_Wl, np.float32), np.asarray(sage_Wr, np.float32),
         np.asarray(gin_W1[0], np.float32), np.asarray(gin_W2[0], np.float32),
         np.asarray(gin_W1[1], np.float32), np.asarray(gin_W2[1], np.float32)],
        axis=1)                                   # [128, 6*128], lhsT layout
    biases = np.stack(
        [np.asarray(sage_bl), np.asarray(gin_b1[0]), np.asarray(gin_b2[0]),
         np.asarray(gin_b1[1]), np.asarray(gin_b2[1])], axis=1).astype(np.float32)
    gamma = np.asarray(bn_gamma, np.float32).T.copy()   # [128, L]
    beta = np.asarray(bn_beta, np.float32).T.copy()
    iota = np.broadcast_to(np.arange(128, dtype=np.float32), (128, 128)).copy()
    ident = np.eye(128, dtype=np.float32)

    in_maps = []
    for c in range(NCORES):
        pc = per_core[c]
        m = {"x_table": x_table, "xT": pc["xT"], "dstl": pc["dstl"],
             "invdeg": pc["invdeg"], "batch_sel": pc["batch_sel"],
             "iota": iota, "ident": ident, "weights": weights,
             "biases": biases, "gamma": gamma, "beta": beta}
        for b in range(NBUCK):
            m[f"gidx{b}"] = pc["gidx"][b]
        in_maps.append(m)

    nc = _build(meta)
    return nc, in_maps, graph_base


TRACE = False
LAST_EXEC_NS = None


def kernel(**inputs):
    global LAST_EXEC_NS
    nc, in_maps, graph_base = prepare(**inputs)
    if TRACE:
        res = run_bass_kernel_spmd(nc, in_maps, list(range(NCORES)), trace=True)
        LAST_EXEC_NS = res.exec_time_ns
    else:
        res = run_bass_kernel_spmd(nc, in_maps, list(range(NCORES)))
    return _unshard(res.results, graph_base)


def _unshard(results, graph_base):
    node_feats = np.concatenate(
        [results[c]["node_feats"][:NC] for c in range(NCORES)], axis=0)
    pooled = np.zeros((NUM_GRAPHS, L * 128), np.float32)
    for c in range(NCORES):
        g0 = graph_base[c]
        nrow = min(128, NUM_GRAPHS - g0)
        pooled[g0:g0 + nrow] += results[c]["pooled"][:nrow]
    return pooled, node_feats
